# revision 1
# baseline (speedup 1.0000x reference)
"""Bass/Tile kernel builder for the bidirectional LSTM (S=512, B=64, I=H=512).

Sharding: 8 cores, each runs BOTH directions on a batch slice of 8.
Per core:
  Phase 1: xproj[d] = x[d] @ W_ih[d].T + b  (big GEMM, weights-stationary)
           -> DRAM ring, layout [d, tok_tile, chunk, 128, 512]
  Phase 2: 512-step recurrence.
    Gate layout (per direction d, col groups gA/gB):
      group holds all four gates' column-chunk: [i c | f c | o c | g c] (256 each)
      fwd: groups 0 (cols 0:256) and 1 (cols 256:512) -> PSUM partitions 0-7, 32-39
      bwd: groups 2, 3 -> partitions 64-71, 96-103
    matmuls: lhsT = hT tile [128, 8] (h transposed, fp32r), rhs = W_hh
      reordered slab [128, 512], col-tiled via out base partition 32g.
    xproj injected via selector matmul (I8 stationary, rhs = xs slab [8,512]).
    ACT: tanh(g), sigmoid(i,f,o) from PSUM; DVE: c/h updates; PE transposes
    h chunks back into hT for the next step.
"""

import sys
if "/opt/trn_rl_repo" not in sys.path:
    sys.path.insert(0, "/opt/trn_rl_repo")
import numpy as np

import concourse.bass as bass
import concourse.bacc as bacc
import concourse.mybir as mybir
import concourse.tile as tile

F32 = mybir.dt.float32
F32R = mybir.dt.float32r
AF = mybir.ActivationFunctionType
ALU = mybir.AluOpType

S, B, I, H = 512, 64, 512, 512
NC = 8
BC = B // NC          # batch per core = 8
G4 = 4 * H            # 2048
CH = H // 2           # 256: gate column chunk per group
TOK_TILE = 128        # phase-1 token tile
N_TOK = S * BC        # 4096 tokens per direction per core
N_TT = N_TOK // TOK_TILE   # 32 token tiles
N_GC = G4 // 512      # 4 gate chunks of 512 in phase-1


def reorder_cols(dirn_mats):
    """Build the reordered gate-column permutation.

    W_hh rows are [i(512) | f(512) | g(512) | o(512)].  We want rhs columns
    ordered per group: group0 = [i0 f0 o0 g0] (chunks cols 0:256 of each
    gate), group1 = [i1 f1 o1 g1].
    Returns an index array perm[2048] such that reordered[:, j] = orig[:, perm[j]].
    """
    idx = []
    for gate in (2, 0, 1, 3):  # g, i, f, o
        idx.extend(range(gate * H, (gate + 1) * H))
    return np.array(idx, dtype=np.int64)


PERM = reorder_cols(None)


def prep_core_inputs(inpt, W_ih_f, W_hh_f, b_ih_f, b_hh_f,
                     W_ih_b, W_hh_b, b_ih_b, b_hh_b):
    """Host-side prep.  Returns (shared dict, per-core list of dicts)."""
    x_f = np.ascontiguousarray(inpt)          # [S, B, I]
    x_b = np.ascontiguousarray(inpt[::-1])    # flipped for backward scan

    shared = {}
    for d, (Wih, Whh, bih, bhh) in (("f", (W_ih_f, W_hh_f, b_ih_f, b_hh_f)),
                                    ("b", (W_ih_b, W_hh_b, b_ih_b, b_hh_b))):
        Wr_ih = np.ascontiguousarray(Wih.T[:, PERM], dtype=np.float32)  # [512,2048]
        Wr_hh = np.ascontiguousarray(Whh.T[:, PERM], dtype=np.float32)  # [512,2048]
        bias = np.ascontiguousarray((bih + bhh)[PERM], dtype=np.float32)[None, :]
        # SBUF slab layout [128, 4, 2048]: partition p, ktile k -> row 128k+p
        shared[f"Wih_{d}"] = np.ascontiguousarray(
            Wr_ih.reshape(4, 128, G4).transpose(1, 0, 2))
        shared[f"Whh_{d}"] = np.ascontiguousarray(
            Wr_hh.reshape(4, 128, G4).transpose(1, 0, 2))
        shared[f"bias_{d}"] = bias                                     # [1, 2048]

    sel_blk = np.zeros((32, 32), dtype=np.float32)
    sel_blk[0:8, 0:8] = np.eye(8, dtype=np.float32)
    shared["sel8"] = np.tile(sel_blk, (4, 1))                          # [128, 32]
    shared["ones1"] = np.ones((1, 128), dtype=np.float32)              # [1, 128]
    shared["ident"] = np.tile(np.eye(8, dtype=np.float32), (16, 1))    # [128, 8]

    in_maps = []
    for c in range(NC):
        bs = slice(c * BC, (c + 1) * BC)
        m = dict(shared)
        for d, x in (("f", x_f), ("b", x_b)):
            xs = x[:, bs, :]                       # [S, 8, I]
            xT = xs.reshape(S * BC, I).T           # [I, S*8] tokens t-major
            m[f"xT_{d}"] = np.ascontiguousarray(xT, dtype=np.float32)
        in_maps.append(m)
    return in_maps


def assemble_output(results):
    """results: list of 8 per-core dicts with out_f/out_b [S, 128, 4, 8]."""
    out = np.empty((S, B, 2 * H), dtype=np.float32)
    for c in range(NC):
        bs = slice(c * BC, (c + 1) * BC)
        for d, off in (("f", 0), ("b", H)):
            slab = results[c][f"out_{d}"]          # [S, 128, 4, 8] = [t, r, k, b]
            # h[t, b, 128k + r] = slab[t, r, k, b]
            h = slab.transpose(0, 3, 2, 1).reshape(S, BC, H)
            out[:, bs, off:off + H] = h.astype(np.float32)
    return out


def build_nc(n_steps=S, interleave=True):
    """Build the full Bass program. Returns nc."""
    nc = bacc.Bacc("TRN2", target_bir_lowering=False, debug=False)

    # ---- DRAM I/O -------------------------------------------------------
    dram = {}
    for d in ("f", "b"):
        dram[f"xT_{d}"] = nc.declare_dram_parameter(
            f"xT_{d}", [I, N_TOK], F32R, isOutput=False)
        dram[f"Wih_{d}"] = nc.declare_dram_parameter(
            f"Wih_{d}", [128, 4, G4], F32R, isOutput=False)
        dram[f"Whh_{d}"] = nc.declare_dram_parameter(
            f"Whh_{d}", [128, 4, G4], F32R, isOutput=False)
        dram[f"bias_{d}"] = nc.declare_dram_parameter(
            f"bias_{d}", [1, G4], F32R, isOutput=False)
        dram[f"out_{d}"] = nc.declare_dram_parameter(
            f"out_{d}", [n_steps, 128, 4, BC], F32R, isOutput=True)
    dram["sel8"] = nc.declare_dram_parameter("sel8", [128, 32], F32R, isOutput=False)
    dram["ones1"] = nc.declare_dram_parameter("ones1", [1, 128], F32R, isOutput=False)
    dram["ident"] = nc.declare_dram_parameter("ident", [128, 8], F32R, isOutput=False)

    # internal xproj ring in DRAM: [d, tok_tile, chunk, 128, 512]
    n_tt = (n_steps * BC + TOK_TILE - 1) // TOK_TILE
    xproj = {d: nc.dram_tensor(f"xproj_{d}", [n_tt, N_GC, TOK_TILE, 512], F32R)
             for d in ("f", "b")}

    DIRS = ("f", "b")
    # partition bases of the 4 col groups: fwd groups 0,1; bwd groups 2,3
    GRP = {"f": (0, 32), "b": (64, 96)}

    with tile.TileContext(nc) as tc:
        with (
            tc.tile_pool(name="weights", bufs=1) as wpool,
            tc.tile_pool(name="consts", bufs=1) as cpool,
            tc.tile_pool(name="p1w", bufs=1) as p1w,
            tc.tile_pool(name="p1x", bufs=2) as p1x,
            tc.tile_pool(name="p1out", bufs=2) as p1out,
            tc.tile_pool(name="p1ps", bufs=2, space="PSUM") as p1ps,
            tc.tile_pool(name="state", bufs=1) as spool,
            tc.tile_pool(name="xs", bufs=2) as xspool,
            tc.tile_pool(name="gps", bufs=2, space="PSUM") as gpspool,
            tc.tile_pool(name="tps", bufs=2, space="PSUM") as tpspool,
            tc.tile_pool(name="eltw", bufs=1) as epool,
        ):
            # ---- resident constants/weights --------------------------------
            Whh_sb = {}
            for d in DIRS:
                Whh_sb[d] = wpool.tile([128, 4, G4], F32R, tag=f"whh{d}", name=f"whh{d}")
                for k in range(4):
                    nc.sync.dma_start(Whh_sb[d][:, k, :], dram[f"Whh_{d}"][:, k, :])
            sel8 = cpool.tile([128, 32], F32R, tag="sel8")
            ones1 = cpool.tile([1, 128], F32R, tag="ones1")
            ident = cpool.tile([128, 8], F32R, tag="ident")
            nc.sync.dma_start(sel8[:, :], dram["sel8"][:, :])
            nc.sync.dma_start(ones1[:, :], dram["ones1"][:, :])
            nc.sync.dma_start(ident[:, :], dram["ident"][:, :])

            # ---- phase 1: xproj = xT.T @ Wih + bias ------------------------
            if True:
                Wih_sb, bias_sb = {}, {}
                for d in DIRS:
                    Wih_sb[d] = p1w.tile([128, 4, G4], F32R, tag=f"wih{d}",
                                         name=f"wih{d}")
                    bias_sb[d] = p1w.tile([1, G4], F32R, tag=f"bias{d}",
                                          name=f"biassb{d}")
                    for k in range(4):
                        nc.sync.dma_start(Wih_sb[d][:, k, :],
                                          dram[f"Wih_{d}"][:, k, :])
                    nc.sync.dma_start(bias_sb[d][:, :], dram[f"bias_{d}"][:, :])
                def emit_p1_tile(d, i):
                    xTd = dram[f"xT_{d}"].rearrange("(k p) t -> p k t", p=128)
                    xt = p1x.tile([128, 4, TOK_TILE], F32R, tag="xt", name=f"xt{d}{i}")
                    nc.sync.dma_start(
                        xt[:, :, :],
                        xTd[:, :, i * TOK_TILE:(i + 1) * TOK_TILE])
                    for c in range(N_GC):
                        ps = p1ps.tile([128, 512], F32, tag="p1ps", name=f"p1ps{d}{i}{c}")
                        for k in range(4):
                            nc.tensor.matmul(
                                ps[:, :],
                                xt[:, k, :],
                                Wih_sb[d][:, k, c * 512:(c + 1) * 512],
                                start=(k == 0), stop=False)
                        nc.tensor.matmul(
                            ps[:, :], ones1[:, :],
                            bias_sb[d][:, c * 512:(c + 1) * 512],
                            start=False, stop=True)
                        xo = p1out.tile([128, 512], F32R, tag="p1o", name=f"p1o{d}{i}{c}")
                        nc.scalar.copy(xo[:, :], ps[:, :])
                        nc.sync.dma_start(xproj[d][i, c, :, :], xo[:, :])

                P1_LOOK = 2
                for i in range(min(P1_LOOK, n_tt)):
                    for d in DIRS:
                        emit_p1_tile(d, i)

            # ---- phase 2: recurrence --------------------------------------
            # No matmul column tiling (walrus limitation): every matmul's
            # output sits at PSUM partitions 0..M.  Per (dir, half) the gates
            # accumulate in their own [32, 1024] PSUM tile; halves of the
            # reordered gate columns: half0 = [g | i], half1 = [f | o].
            hT = {d: [spool.tile([128, 4 * BC], F32R, tag=f"hT{d}{j}", name=f"hT{d}{j}")
                      for j in range(2)] for d in DIRS}
            cst = {d: [spool.tile([BC, H], F32, tag=f"c{d}{j}", name=f"cst{d}{j}")
                       for j in range(2)] for d in DIRS}
            for d in DIRS:
                nc.vector.memset(hT[d][0][:, :].bitcast(F32), 0.0)
                nc.vector.memset(cst[d][0][:, :], 0.0)

            for t in range(n_steps):
                cur, nxt = t % 2, (t + 1) % 2
                if t % 16 == 0:
                    nxt_tile = t // 16 + P1_LOOK
                    if nxt_tile < n_tt:
                        for d in DIRS:
                            emit_p1_tile(d, nxt_tile)
                # xs slabs: [128, 512] per dir; chunk c at partitions 32c
                xs = {}
                for d in DIRS:
                    xs[d] = xspool.tile([128, 512], F32R, tag=f"xs{d}",
                                        name=f"xs{d}")
                    tt, tr = (t * BC) // TOK_TILE, (t * BC) % TOK_TILE
                    for c in range(4):
                        nc.sync.dma_start(
                            xs[d][32 * c:32 * c + BC, :],
                            xproj[d][tt, c, tr:tr + BC, :])

                gh = {}
                for d in DIRS:
                    for half in range(2):
                        g = gpspool.tile([32, 1024], F32, tag="gh",
                                         name=f"gh{d}{half}")
                        gh[(d, half)] = g
                        for q in range(2):        # two 512-col quarters
                            c = half * 2 + q
                            o32 = g[0:32, q * 512:(q + 1) * 512]
                            nc.tensor.matmul(
                                o32, sel8[32 * c:32 * c + BC, :],
                                xs[d][32 * c:32 * c + BC, :],
                                start=True, stop=False,
                                skip_group_check=True,
                                tile_position=(32 * c, 0))
                            o = g[0:BC, q * 512:(q + 1) * 512]
                            for k in range(4):
                                nc.tensor.matmul(
                                    o, hT[d][cur][:, k * BC:(k + 1) * BC],
                                    Whh_sb[d][:, k, c * 512:(c + 1) * 512],
                                    start=False, stop=(k == 3),
                                    skip_group_check=True)

                for d in DIRS:
                    g0, g1 = gh[(d, 0)], gh[(d, 1)]
                    tg = epool.tile([BC, H], F32, tag=f"tg{d}", name=f"tg{d}")
                    si = epool.tile([BC, H], F32, tag=f"si{d}", name=f"si{d}")
                    sf = epool.tile([BC, H], F32, tag=f"sf{d}", name=f"sf{d}")
                    so = epool.tile([BC, H], F32, tag=f"so{d}", name=f"so{d}")
                    nc.scalar.activation(tg[:, :], g0[0:BC, 0:512], AF.Tanh)
                    nc.scalar.activation(si[:, :], g0[0:BC, 512:1024], AF.Sigmoid)
                    nc.scalar.activation(sf[:, :], g1[0:BC, 0:512], AF.Sigmoid)
                    nc.scalar.activation(so[:, :], g1[0:BC, 512:1024], AF.Sigmoid)

                    ig = epool.tile([BC, H], F32, tag=f"ig{d}", name=f"ig{d}")
                    fc = epool.tile([BC, H], F32, tag=f"fc{d}", name=f"fc{d}")
                    nc.vector.tensor_mul(ig[:, :], si[:, :], tg[:, :])
                    nc.vector.tensor_mul(fc[:, :], sf[:, :], cst[d][cur][:, :])
                    nc.vector.tensor_add(cst[d][nxt][:, :], ig[:, :], fc[:, :])
                    tc_t = epool.tile([BC, H], F32, tag=f"tc{d}", name=f"tc{d}")
                    nc.scalar.activation(tc_t[:, :], cst[d][nxt][:, :], AF.Tanh)
                    # staggered tail: per 128-col chunk k, the h multiply,
                    # transpose, and hT copy land independently so the next
                    # step's Ktile-k matmul unblocks as soon as chunk k is in.
                    ht = epool.tile([BC, H], F32R, tag=f"ht{d}", name=f"ht{d}")
                    pt = tpspool.tile([128, 4 * BC], F32R, tag="pt",
                                      name=f"pt{d}")
                    for k in range(4):
                        nc.vector.tensor_mul(ht[:, k * 128:(k + 1) * 128],
                                             so[:, k * 128:(k + 1) * 128],
                                             tc_t[:, k * 128:(k + 1) * 128])
                        nc.tensor.matmul(
                            pt[:, k * BC:(k + 1) * BC],
                            ht[:, k * 128:(k + 1) * 128],
                            ident[0:BC, :],
                            start=(k == 0), stop=(k == 3),
                            is_transpose=True,
                            skip_group_check=True)
                        nc.vector.tensor_copy(
                            hT[d][nxt][:, k * BC:(k + 1) * BC],
                            pt[:, k * BC:(k + 1) * BC])
                    nc.sync.dma_start(
                        dram[f"out_{d}"][t, :, :, :],
                        hT[d][nxt][:, :].rearrange("p (k b) -> p k b", k=4))

    nc.compile()
    return nc

# ---------------------------------------------------------------------------
# Entry point: kernel(**inputs) -> np.ndarray  [S, B, 2H]
# ---------------------------------------------------------------------------
from concourse.bass_utils import run_bass_kernel_spmd

_NC_CACHE = {}


def _get_nc():
    if "nc" not in _NC_CACHE:
        _NC_CACHE["nc"] = build_nc(n_steps=S)
    return _NC_CACHE["nc"]


def kernel(**inputs):
    nc = _get_nc()
    in_maps = prep_core_inputs(**inputs)
    res = run_bass_kernel_spmd(nc, in_maps, list(range(NC)))
    return assemble_output(res.results)

